# revision 1
# baseline (speedup 1.0000x reference)
"""MoE (top-2 of 8 experts) Trainium2 kernel.

Strategy (expert parallelism, matching the sharding hint):
  - Host computes the gate (tiny: [4096,1024]@[1024,8]), top-2 routing and
    combine weights, then dispatches tokens to experts ("all-to-all" done as
    host-side sharding: core e receives the tokens routed to expert e plus
    expert e's weights).
  - Each of the 8 cores runs a dense 2-layer FFN on its token batch:
      yT = w2.T @ relu(w1.T @ xT + b1)
    with everything kept transposed (feature dim on partitions) so both
    matmuls contract on the partition axis with weights as the stationary
    operand. All matmul inputs are float32r: full fp32 storage, ~1.5e-4
    matmul relative error at full (bf16-rate) PE speed.
  - Host gathers per-core outputs and scatter-adds cw * (y + b2) into the
    final [B,S,D] output.

Hardcoded problem shape: B=4, S=1024, D=1024, F=4096, E=8, TOP_K=2.
"""

import numpy as np

import concourse.bass as bass
import concourse.mybir as mybir
import concourse.tile as tile
from concourse import bacc
from concourse.bass_utils import run_bass_kernel_spmd

B, S, D, F, E = 4, 1024, 1024, 4096, 8
TOP_K = 2
P = 128
DC = D // P   # 8 d-chunks
FC = F // P   # 32 f-chunks

_program_cache: dict = {}


def _split(n, max_piece):
    """Split n (multiple of 128) into pieces <= max_piece, all multiples of 128."""
    k = -(-n // max_piece)
    base = 128 * (n // 128 // k)
    pieces = [base] * k
    rem = n - base * k
    i = 0
    while rem > 0:
        pieces[i] += 128
        rem -= 128
        i += 1
    return pieces


def _build_program(cap):
    """One SPMD program, identical on all cores: dense FFN on [cap] tokens."""
    batches = _split(cap, 768)       # token super-batches resident in SBUF
    f32r = mybir.dt.float32r
    f32 = mybir.dt.float32

    nc = bacc.Bacc("TRN2", target_bir_lowering=False, debug=False, num_devices=E)
    xT = nc.dram_tensor("xT", [DC, P, cap], f32r, kind="ExternalInput")
    w1p = nc.dram_tensor("w1p", [FC, P, D], f32r, kind="ExternalInput")
    w2p = nc.dram_tensor("w2p", [DC, P, F], f32r, kind="ExternalInput")
    b1p = nc.dram_tensor("b1p", [P, FC], f32, kind="ExternalInput")
    yT = nc.dram_tensor("yT", [DC, P, cap], f32, kind="ExternalOutput")

    tb_max = max(batches)
    with tile.TileContext(nc) as tc:
        with (
            tc.tile_pool(name="consts", bufs=1) as consts,
            tc.tile_pool(name="xp", bufs=1) as xp,
            tc.tile_pool(name="hp", bufs=1) as hp,
            tc.tile_pool(name="w1pool", bufs=3) as w1pool,
            tc.tile_pool(name="w2pool", bufs=2) as w2pool,
            tc.tile_pool(name="yp", bufs=3) as yp,
            tc.tile_pool(name="psh", bufs=2, space="PSUM") as psh,
            tc.tile_pool(name="psy", bufs=2, space="PSUM") as psy,
        ):
            b1_sb = consts.tile([P, FC], f32)
            nc.sync.dma_start(b1_sb[:], b1p[:])

            t0 = 0
            for tsz in batches:
                chunks = _split(tsz, 512)
                x_sb = xp.tile([P, DC, tb_max], f32r, tag="x")
                for dc in range(DC):
                    nc.sync.dma_start(x_sb[:, dc, :tsz], xT[dc, :, t0:t0 + tsz])

                h_sb = hp.tile([P, FC, tb_max], f32r, tag="h")

                # layer 1: hT[fc] = relu(sum_dc w1[dc,fc].T @ xT[dc] + b1[fc])
                for fc in range(FC):
                    w1_sb = w1pool.tile([P, D], f32r, tag="w1")
                    nc.sync.dma_start(w1_sb[:], w1p[fc])
                    c0 = 0
                    for csz in chunks:
                        ph = psh.tile([P, 512], f32, tag="ph")
                        for dc in range(DC):
                            nc.tensor.matmul(
                                ph[:, :csz],
                                w1_sb[:, dc * P:(dc + 1) * P],
                                x_sb[:, dc, c0:c0 + csz],
                                start=(dc == 0), stop=(dc == DC - 1),
                            )
                        nc.scalar.activation(
                            h_sb[:, fc, c0:c0 + csz], ph[:, :csz],
                            mybir.ActivationFunctionType.Relu,
                            bias=b1_sb[:, fc:fc + 1],
                        )
                        c0 += csz

                # layer 2: yT[dc] = sum_fc w2[fc,dc].T @ hT[fc]
                for dc in range(DC):
                    w2_sb = w2pool.tile([P, F], f32r, tag="w2")
                    nc.sync.dma_start(w2_sb[:], w2p[dc])
                    c0 = 0
                    for csz in chunks:
                        py = psy.tile([P, 512], f32, tag="py")
                        for fc in range(FC):
                            nc.tensor.matmul(
                                py[:, :csz],
                                w2_sb[:, fc * P:(fc + 1) * P],
                                h_sb[:, fc, c0:c0 + csz],
                                start=(fc == 0), stop=(fc == FC - 1),
                            )
                        y_sb = yp.tile([P, 512], f32, tag="y")
                        nc.vector.tensor_copy(y_sb[:, :csz], py[:, :csz])
                        nc.sync.dma_start(
                            yT[dc, :, t0 + c0:t0 + c0 + csz], y_sb[:, :csz]
                        )
                        c0 += csz
                t0 += tsz
    nc.finalize()
    return nc


def _route(x2d, gate_w, gate_b):
    """Host gate: softmax top-2 routing. Returns per-expert index lists and
    combine weights."""
    logits = (x2d @ gate_w + gate_b).astype(np.float64)
    logits -= logits.max(axis=-1, keepdims=True)
    p = np.exp(logits)
    p /= p.sum(axis=-1, keepdims=True)
    order = np.argsort(-p, axis=-1)[:, :TOP_K]
    idx = []
    cw = []
    for e in range(E):
        sel = np.nonzero((order == e).any(axis=-1))[0]
        idx.append(sel)
        cw.append(p[sel, e].astype(np.float32))
    return idx, cw


def kernel(x, gate_w, gate_b, w1, b1, w2, b2, _run_kwargs=None, _out=None):
    x = np.asarray(x, np.float32)
    gate_w = np.asarray(gate_w, np.float32)
    gate_b = np.asarray(gate_b, np.float32)
    w1 = np.asarray(w1, np.float32)
    b1 = np.asarray(b1, np.float32)
    w2 = np.asarray(w2, np.float32)
    b2 = np.asarray(b2, np.float32)

    x2d = x.reshape(-1, D)
    idx, cw = _route(x2d, gate_w, gate_b)

    cap = max(256, -(-max(len(i) for i in idx) // P) * P)
    if cap not in _program_cache:
        _program_cache[cap] = _build_program(cap)
    nc = _program_cache[cap]

    in_maps = []
    for e in range(E):
        n_e = len(idx[e])
        xe = np.zeros((cap, D), np.float32)
        xe[:n_e] = x2d[idx[e]]
        # xT[dc, p, t] = xe[t, dc*128+p]
        xT = np.ascontiguousarray(xe.T.reshape(DC, P, cap))
        # w1p[fc, p, dc*128+q] = w1[e][dc*128+p, fc*128+q]
        w1p = np.ascontiguousarray(
            w1[e].reshape(DC, P, FC, P).transpose(2, 1, 0, 3).reshape(FC, P, D)
        )
        # w2p[dc, p, fc*128+q] = w2[e][fc*128+p, dc*128+q]
        w2p = np.ascontiguousarray(
            w2[e].reshape(FC, P, DC, P).transpose(2, 1, 0, 3).reshape(DC, P, F)
        )
        b1p = np.ascontiguousarray(b1[e].reshape(FC, P).T)
        in_maps.append({"xT": xT, "w1p": w1p, "w2p": w2p, "b1p": b1p})

    res = run_bass_kernel_spmd(
        nc, in_maps, core_ids=list(range(E)), **(_run_kwargs or {})
    )
    if _out is not None:
        _out.append(res)

    out = np.zeros((B * S, D), np.float32)
    for e in range(E):
        n_e = len(idx[e])
        # yT [DC, P, cap] -> y [cap, D]
        y = res.results[e]["yT"].transpose(2, 0, 1).reshape(cap, D)[:n_e]
        out[idx[e]] += cw[e][:, None] * (y + b2[e])
    return out.reshape(B, S, D)


# revision 5
# speedup vs baseline: 1.1782x; 1.1782x over previous
"""MoE (top-2 of 8 experts) Trainium2 kernel.

Strategy (expert parallelism, matching the sharding hint):
  - Host computes the gate (tiny: [4096,1024]@[1024,8]), top-2 routing and
    combine weights, then dispatches tokens to experts ("all-to-all" done as
    host-side sharding: core e receives the tokens routed to expert e plus
    expert e's weights).
  - Each of the 8 cores runs a dense 2-layer FFN on its token batch:
      yT = w2.T @ relu(w1.T @ xT + b1)
    with everything kept transposed (feature dim on partitions) so both
    matmuls contract on the partition axis with weights as the stationary
    operand. All matmul inputs are float32r: full fp32 storage, ~1.5e-4
    matmul relative error at full (bf16-rate) PE speed.
  - Host gathers per-core outputs and scatter-adds cw * (y + b2) into the
    final [B,S,D] output.

Hardcoded problem shape: B=4, S=1024, D=1024, F=4096, E=8, TOP_K=2.
"""

import numpy as np

import concourse.bass as bass
import concourse.mybir as mybir
import concourse.tile as tile
from concourse import bacc
from concourse.bass_utils import run_bass_kernel_spmd

B, S, D, F, E = 4, 1024, 1024, 4096, 8
TOP_K = 2
P = 128
DC = D // P   # 8 d-chunks
FC = F // P   # 32 f-chunks

_program_cache: dict = {}


def _split(n, max_piece):
    """Split n (multiple of 128) into pieces <= max_piece, all multiples of 128."""
    k = -(-n // max_piece)
    base = 128 * (n // 128 // k)
    pieces = [base] * k
    rem = n - base * k
    i = 0
    while rem > 0:
        pieces[i] += 128
        rem -= 128
        i += 1
    return pieces


def _build_program(cap, reps=1):
    """One SPMD program, identical on all cores: dense FFN on [cap] tokens.
    reps>1 repeats the whole computation (benchmarking only)."""
    batches = _split(cap, 768)       # token super-batches resident in SBUF
    f32r = mybir.dt.float32r
    f32 = mybir.dt.float32

    nc = bacc.Bacc("TRN2", target_bir_lowering=False, debug=False, num_devices=E)
    xT = nc.dram_tensor("xT", [DC, P, cap], f32r, kind="ExternalInput")
    w1p = nc.dram_tensor("w1p", [FC, P, D], f32r, kind="ExternalInput")
    w2p = nc.dram_tensor("w2p", [DC, P, F], f32r, kind="ExternalInput")
    b1p = nc.dram_tensor("b1p", [P, FC], f32, kind="ExternalInput")
    yT = nc.dram_tensor("yT", [DC, P, cap], f32, kind="ExternalOutput")

    tb_max = max(batches)
    with tile.TileContext(nc) as tc:
        with (
            tc.tile_pool(name="consts", bufs=1) as consts,
            tc.tile_pool(name="xp", bufs=1) as xp,
            tc.tile_pool(name="hp", bufs=1) as hp,
            tc.tile_pool(name="w1pool", bufs=3) as w1pool,
            tc.tile_pool(name="w2pool", bufs=2) as w2pool,
            tc.tile_pool(name="yp", bufs=3) as yp,
            tc.tile_pool(name="psh", bufs=2, space="PSUM") as psh,
            tc.tile_pool(name="psy", bufs=2, space="PSUM") as psy,
        ):
            b1_sb = consts.tile([P, FC], f32)
            nc.sync.dma_start(b1_sb[:], b1p[:])

            for _rep in range(reps):
                t0 = 0
                for tsz in batches:
                    chunks = _split(tsz, 512)
                    x_sb = xp.tile([P, DC, tb_max], f32r, tag="x")
                    for dc in range(DC):
                        nc.sync.dma_start(x_sb[:, dc, :tsz], xT[dc, :, t0:t0 + tsz])

                    h_sb = hp.tile([P, FC, tb_max], f32r, tag="h")

                    # layer 1: hT[fc] = relu(sum_dc w1[dc,fc].T @ xT[dc] + b1[fc])
                    for fc in range(FC):
                        w1_sb = w1pool.tile([P, D], f32r, tag="w1")
                        nc.sync.dma_start(w1_sb[:], w1p[fc])
                        c0 = 0
                        for csz in chunks:
                            ph = psh.tile([P, 512], f32, tag="ph")
                            for dc in range(DC):
                                nc.tensor.matmul(
                                    ph[:, :csz],
                                    w1_sb[:, dc * P:(dc + 1) * P],
                                    x_sb[:, dc, c0:c0 + csz],
                                    start=(dc == 0), stop=(dc == DC - 1),
                                )
                            nc.scalar.activation(
                                h_sb[:, fc, c0:c0 + csz], ph[:, :csz],
                                mybir.ActivationFunctionType.Relu,
                                bias=b1_sb[:, fc:fc + 1],
                            )
                            c0 += csz

                    # layer 2: yT[dc] = sum_fc w2[fc,dc].T @ hT[fc]
                    for dc in range(DC):
                        w2_sb = w2pool.tile([P, F], f32r, tag="w2")
                        nc.sync.dma_start(w2_sb[:], w2p[dc])
                        c0 = 0
                        for csz in chunks:
                            py = psy.tile([P, 512], f32, tag="py")
                            for fc in range(FC):
                                nc.tensor.matmul(
                                    py[:, :csz],
                                    w2_sb[:, fc * P:(fc + 1) * P],
                                    h_sb[:, fc, c0:c0 + csz],
                                    start=(fc == 0), stop=(fc == FC - 1),
                                )
                            y_sb = yp.tile([P, 512], f32, tag="y")
                            nc.vector.tensor_copy(y_sb[:, :csz], py[:, :csz])
                            nc.sync.dma_start(
                                yT[dc, :, t0 + c0:t0 + c0 + csz], y_sb[:, :csz]
                            )
                            c0 += csz
                    t0 += tsz
    nc.finalize()
    return nc


def _route(x2d, gate_w, gate_b):
    """Host gate: softmax top-2 routing. Returns per-expert index lists and
    combine weights."""
    logits = (x2d @ gate_w + gate_b).astype(np.float64)
    logits -= logits.max(axis=-1, keepdims=True)
    p = np.exp(logits)
    p /= p.sum(axis=-1, keepdims=True)
    order = np.argsort(-p, axis=-1)[:, :TOP_K]
    idx = []
    cw = []
    for e in range(E):
        sel = np.nonzero((order == e).any(axis=-1))[0]
        idx.append(sel)
        cw.append(p[sel, e].astype(np.float32))
    return idx, cw


def _pack_inputs(x2d, idx, w1, b1, w2, cap):
    in_maps = []
    for e in range(E):
        n_e = len(idx[e])
        xe = np.zeros((cap, D), np.float32)
        xe[:n_e] = x2d[idx[e]]
        # xT[dc, p, t] = xe[t, dc*128+p]
        xT = np.ascontiguousarray(xe.T.reshape(DC, P, cap))
        # w1p[fc, p, dc*128+q] = w1[e][dc*128+p, fc*128+q]
        w1p = np.ascontiguousarray(
            w1[e].reshape(DC, P, FC, P).transpose(2, 1, 0, 3).reshape(FC, P, D)
        )
        # w2p[dc, p, fc*128+q] = w2[e][fc*128+p, dc*128+q]
        w2p = np.ascontiguousarray(
            w2[e].reshape(FC, P, DC, P).transpose(2, 1, 0, 3).reshape(DC, P, F)
        )
        b1p = np.ascontiguousarray(b1[e].reshape(FC, P).T)
        in_maps.append({"xT": xT, "w1p": w1p, "w2p": w2p, "b1p": b1p})
    return in_maps


def kernel(x, gate_w, gate_b, w1, b1, w2, b2, _run_kwargs=None, _out=None):
    x = np.asarray(x, np.float32)
    gate_w = np.asarray(gate_w, np.float32)
    gate_b = np.asarray(gate_b, np.float32)
    w1 = np.asarray(w1, np.float32)
    b1 = np.asarray(b1, np.float32)
    w2 = np.asarray(w2, np.float32)
    b2 = np.asarray(b2, np.float32)

    x2d = x.reshape(-1, D)
    idx, cw = _route(x2d, gate_w, gate_b)

    cap = max(256, -(-max(len(i) for i in idx) // P) * P)
    if cap not in _program_cache:
        _program_cache[cap] = _build_program(cap)
    nc = _program_cache[cap]

    in_maps = _pack_inputs(x2d, idx, w1, b1, w2, cap)

    res = run_bass_kernel_spmd(
        nc, in_maps, core_ids=list(range(E)), **(_run_kwargs or {})
    )
    if _out is not None:
        _out.append(res)

    out = np.zeros((B * S, D), np.float32)
    for e in range(E):
        n_e = len(idx[e])
        # yT [DC, P, cap] -> y [cap, D]
        y = res.results[e]["yT"].transpose(2, 0, 1).reshape(cap, D)[:n_e]
        out[idx[e]] += cw[e][:, None] * (y + b2[e])
    return out.reshape(B, S, D)
